# revision 15
# baseline (speedup 1.0000x reference)
"""Gemma2 attention (B=2, S=2048, HID=2304, H=8, KVH=4, D=256, window=1024,
softcap=50) on 8 TRN2 NeuronCores.

Sharding: DP2 (batch) x TP4 (heads). Core c -> batch c//4, TP rank r=c%4 with
Q heads {2r, 2r+1} and KV head r (GQA-aligned). Wo is row-split over the head
dim; the 4 partial outputs per batch are summed on the host.

Device kernel (identical program on all cores, fp16 matmuls / fp32 PSUM):
  - Projections are interleaved with attention per 512-token chunk; QT/KT are
    feature-major with RoPE fused into the PSUM->SBUF eviction, V token-major.
  - Attention per query block qi (key blocks [qi-8, qi] cover the causal
    sliding window): scores -> softcap tanh (ACT) -> additive triangular masks
    on the boundary blocks (DVE) -> exp(50t-50) with fused row-sum (ACT
    accum_out; no rowmax since P is float32r, which keeps fp32 range) ->
    PE-transpose P -> P.T @ V; 1/rowsum is folded into the AV eviction.
  - The PE stream is software-pipelined: the previous block's Wo matmuls are
    emitted between this block's score matmuls and its softmax-dependent
    transposes, so the in-order PE never waits on ACT/DVE.
"""
import numpy as np

H, KVH, D = 8, 4, 256
S, HID = 2048, 2304
B = 2
SCALING = 256.0 ** -0.5
SOFTCAP = 50.0
THETA = 10000.0
WINDOW = 1024

P = 128
KC = HID // P            # 18 contraction chunks for projections
NQB = S // P             # 16 query blocks
NTC = 4                  # token chunks for projections
TCW = S // NTC           # 512
WBLK = WINDOW // P       # 8: kj in [qi-WBLK, qi]
HG_WIDTHS = [512, 512, 512, 512, 256]   # 2304 split for Wo output groups

_CACHED = {}


def _build_nc():
    import concourse.bass as bass
    import concourse.mybir as mybir
    import concourse.tile as tile
    from concourse import bacc
    from concourse.masks import make_identity

    f32 = mybir.dt.float32
    f16 = mybir.dt.float16
    f32r = mybir.dt.float32r
    AF = mybir.ActivationFunctionType

    nc = bacc.Bacc(None, target_bir_lowering=False)

    hT = nc.dram_tensor("hT", [HID, S], f16, kind="ExternalInput")
    wqT = nc.dram_tensor("wqT", [HID, 2 * D], f16, kind="ExternalInput")
    wkT = nc.dram_tensor("wkT", [HID, D], f16, kind="ExternalInput")
    wvT = nc.dram_tensor("wvT", [HID, D], f16, kind="ExternalInput")
    woT = nc.dram_tensor("woT", [2 * D, HID], f16, kind="ExternalInput")
    cosT = nc.dram_tensor("cosT", [P, S], f16, kind="ExternalInput")
    sinT = nc.dram_tensor("sinT", [P, S], f16, kind="ExternalInput")
    out = nc.dram_tensor("out", [S, HID], f32, kind="ExternalOutput")

    hTr = hT.rearrange("(c p) s -> p c s", p=P)
    wqTr = wqT.rearrange("(c p) m -> p c m", p=P)
    wkTr = wkT.rearrange("(c p) m -> p c m", p=P)
    wvTr = wvT.rearrange("(c p) m -> p c m", p=P)
    woTr = woT.rearrange("(c p) m -> p c m", p=P)

    with tile.TileContext(nc) as tc:
        with (
            tc.tile_pool(name="wpool", bufs=1) as wpool,
            tc.tile_pool(name="hpool", bufs=2) as hpool,
            tc.tile_pool(name="qkv", bufs=1) as qkv,
            tc.tile_pool(name="work", bufs=3) as work,
            tc.tile_pool(name="att3", bufs=3) as att3,
            tc.tile_pool(name="sc", bufs=4) as scpool,
            tc.tile_pool(name="psA", bufs=8, space="PSUM") as psA,
        ):
            # ---------------- persistent SBUF ----------------
            wq_sb = wpool.tile([P, KC, 2 * D], f16)
            wk_sb = wpool.tile([P, KC, D], f16)
            wv_sb = wpool.tile([P, KC, D], f16)
            wo_sb = wpool.tile([P, 4, HID], f16)
            cos_sb = wpool.tile([P, S], f16)
            sin_sb = wpool.tile([P, S], f16)
            ident16 = wpool.tile([P, P], f16)
            ident32 = wpool.tile([P, P], f32)
            identR = wpool.tile([P, P], f32r)
            mask_edge = wpool.tile([P, P], f32)
            mask_diag = wpool.tile([P, P], f32)
            negcap = wpool.tile([P, 1], f32)
            nc.gpsimd.memset(negcap[:], -SOFTCAP)

            qt_sb = qkv.tile([P, 4, S], f16)    # QT feature-major
            kt_sb = qkv.tile([P, 2, S], f16)    # KT feature-major
            v_sb = qkv.tile([P, NQB, D], f16)   # V token-major

            # DMA: few large descriptors; first chunk's operands first.
            ht0 = hpool.tile([P, KC, TCW], f16, tag="ht", name="ht0")
            nc.sync.dma_start(ht0[:, 0:2, :], hTr[:, 0:2, 0:TCW])
            nc.sync.dma_start(wq_sb[:, 0:2, :], wqTr[:, 0:2, :])
            nc.sync.dma_start(ht0[:, 2:6, :], hTr[:, 2:6, 0:TCW])
            nc.sync.dma_start(wq_sb[:, 2:6, :], wqTr[:, 2:6, :])
            nc.sync.dma_start(ht0[:, 6:12, :], hTr[:, 6:12, 0:TCW])
            nc.sync.dma_start(wq_sb[:, 6:12, :], wqTr[:, 6:12, :])
            nc.sync.dma_start(ht0[:, 12:KC, :], hTr[:, 12:KC, 0:TCW])
            nc.sync.dma_start(wq_sb[:, 12:KC, :], wqTr[:, 12:KC, :])
            nc.sync.dma_start(wk_sb[:], wkTr[:, :, :])
            nc.sync.dma_start(wv_sb[:], wvTr[:, :, :])
            nc.sync.dma_start(cos_sb[:], cosT[:, :])
            nc.sync.dma_start(sin_sb[:], sinT[:, :])
            nc.sync.dma_start(wo_sb[:], woTr[:, :, :])

            make_identity(nc, ident16[:])
            make_identity(nc, ident32[:])
            nc.vector.tensor_copy(identR[:], ident32[:])
            # additive masks: 0 where allowed, -3 where masked (t in [-1,1],
            # exp(50*(t-3)-50) underflows to exactly 0 in fp32)
            nc.gpsimd.memset(mask_edge[:], 0.0)
            nc.gpsimd.affine_select(   # window edge: keep dj - di - 1 >= 0
                out=mask_edge[:], in_=mask_edge[:],
                compare_op=mybir.AluOpType.is_ge, fill=-3.0,
                base=-1, pattern=[[1, P]], channel_multiplier=-1)
            nc.gpsimd.memset(mask_diag[:], 0.0)
            nc.gpsimd.affine_select(   # causal diag: keep di - dj >= 0
                out=mask_diag[:], in_=mask_diag[:],
                compare_op=mybir.AluOpType.is_ge, fill=-3.0,
                base=0, pattern=[[-1, P]], channel_multiplier=1)

            def rope_pair(ps_lo, ps_hi, dst, m_lo, m_hi, ts):
                tsl = slice(ts * TCW, (ts + 1) * TCW)
                cs, sn = cos_sb[:, tsl], sin_sb[:, tsl]
                t1 = work.tile([P, TCW], f16, tag="rope_t1")
                t2 = work.tile([P, TCW], f16, tag="rope_t2")
                nc.vector.tensor_mul(t1[:], ps_hi[:], sn)
                nc.vector.tensor_mul(t2[:], ps_lo[:], sn)
                lo = dst[:, m_lo, tsl]
                hi = dst[:, m_hi, tsl]
                nc.vector.tensor_mul(lo, ps_lo[:], cs)
                nc.vector.tensor_sub(lo, lo, t1[:])
                nc.vector.tensor_mul(hi, ps_hi[:], cs)
                nc.vector.tensor_add(hi, hi, t2[:])

            def proj_chunk(ts, ht):
                for pair in range(2):
                    pq = [psA.tile([P, 512], f32, tag="bank",
                                   name=f"pq{ts}_{pair}_{i}") for i in range(2)]
                    for i in range(2):
                        m = 2 * pair + i
                        for k in range(KC):
                            nc.tensor.matmul(
                                pq[i][:], wq_sb[:, k, m * P:(m + 1) * P],
                                ht[:, k, :], start=(k == 0), stop=(k == KC - 1))
                    rope_pair(pq[0], pq[1], qt_sb, 2 * pair, 2 * pair + 1, ts)
                pk = [psA.tile([P, 512], f32, tag="bank", name=f"pk{ts}_{i}")
                      for i in range(2)]
                for i in range(2):
                    for k in range(KC):
                        nc.tensor.matmul(
                            pk[i][:], wk_sb[:, k, i * P:(i + 1) * P],
                            ht[:, k, :], start=(k == 0), stop=(k == KC - 1))
                rope_pair(pk[0], pk[1], kt_sb, 0, 1, ts)
                for mt in range(4):
                    pv = psA.tile([P, 512], f32, tag="bank")
                    for k in range(KC):
                        nc.tensor.matmul(
                            pv[:, :D], ht[:, k, mt * P:(mt + 1) * P],
                            wv_sb[:, k, :], start=(k == 0), stop=(k == KC - 1))
                    nc.scalar.copy(v_sb[:, ts * 4 + mt, :], pv[:, :D])

            def emit_wo(prev):
                """Wo partial for the previous query block (5 psum groups)."""
                if prev is None:
                    return
                atT, q0 = prev
                osb = work.tile([P, HID], f32, tag="osb", name=f"osb{q0}")
                hg0 = 0
                for gi, hgw in enumerate(HG_WIDTHS):
                    po = psA.tile([P, 512], f32, tag="bank",
                                  name=f"po{q0}_{gi}")
                    for m in range(4):
                        nc.tensor.matmul(
                            po[:, :hgw], atT[:, m, :],
                            wo_sb[:, m, hg0:hg0 + hgw],
                            start=(m == 0), stop=(m == 3))
                    if gi % 2 == 0:
                        nc.vector.tensor_copy(osb[:, hg0:hg0 + hgw],
                                              po[:, :hgw])
                    else:
                        nc.scalar.copy(osb[:, hg0:hg0 + hgw], po[:, :hgw])
                    hg0 += hgw
                nc.sync.dma_start(out[q0:q0 + P, :], osb[:])

            def attn_block(qi, prev):
                kj0 = max(0, qi - WBLK)
                nkb = qi - kj0 + 1
                nk = nkb * P
                qsl = slice(qi * P, (qi + 1) * P)
                at_qi = work.tile([P, 2 * D], f16, tag="at_qi")
                pav = psA.tile([P, 512], f32, tag="bank", name=f"pav{qi}")

                # scores + tanh for both heads first (independent PE work)
                tbufs = []
                for h in range(2):
                    tbuf = scpool.tile([P, 9 * P], f32, tag="tbuf",
                                       name=f"tbuf{qi}_{h}")
                    for g0 in range(0, nk, 512):
                        gw = min(512, nk - g0)
                        ps = psA.tile([P, 512], f32, tag="bank",
                                      name=f"ps{qi}_{h}_{g0}")
                        ksl = slice(kj0 * P + g0, kj0 * P + g0 + gw)
                        for i in range(2):
                            nc.tensor.matmul(
                                ps[:, :gw], qt_sb[:, 2 * h + i, qsl],
                                kt_sb[:, i, ksl], start=(i == 0), stop=(i == 1))
                        nc.scalar.activation(
                            tbuf[:, g0:g0 + gw], ps[:, :gw], AF.Tanh,
                            scale=SCALING / SOFTCAP)
                    tbufs.append(tbuf)

                # previous block's Wo fills this block's softmax latency
                emit_wo(prev)

                for h in range(2):
                    tbuf = tbufs[h]
                    if kj0 == qi - WBLK:
                        nc.vector.tensor_add(tbuf[:, :P], tbuf[:, :P],
                                             mask_edge[:])
                    dsl = slice((nkb - 1) * P, nkb * P)
                    nc.vector.tensor_add(tbuf[:, dsl], tbuf[:, dsl],
                                         mask_diag[:])
                    negm = scpool.tile([P, 1], f32, tag="negm")
                    nc.vector.tensor_reduce(
                        out=negm[:], in_=tbuf[:, :nk], op=mybir.AluOpType.max,
                        axis=mybir.AxisListType.X, negate=True)
                    negm50 = scpool.tile([P, 1], f32, tag="negm50")
                    nc.vector.tensor_scalar_mul(negm50[:], negm[:], SOFTCAP)
                    pbuf = scpool.tile([P, 9 * P], f16, tag="pbuf",
                                       name=f"pbuf{qi}_{h}")
                    sums = scpool.tile([P, 1], f32, tag="sums")
                    nc.scalar.activation(
                        pbuf[:, :nk], tbuf[:, :nk], AF.Exp,
                        bias=negm50[:], scale=SOFTCAP, accum_out=sums[:])
                    recip = scpool.tile([P, 1], f32, tag="recip")
                    nc.vector.reciprocal(recip[:], sums[:])
                    pt = work.tile([P, 9, P], f16, tag="pt")
                    nb = 0
                    for b0 in range(0, nkb, 4):
                        bw = min(4, nkb - b0)
                        ptp = psA.tile([P, 512], f16, tag="bank",
                                       name=f"ptp{qi}_{h}_{b0}")
                        for j in range(bw):
                            nc.tensor.transpose(
                                ptp[:, j * P:(j + 1) * P],
                                pbuf[:, (b0 + j) * P:(b0 + j + 1) * P],
                                ident16[:])
                        if nb % 2 == 0:
                            nc.vector.tensor_copy(
                                pt[:, b0:b0 + bw, :], ptp[:, :bw * P])
                        else:
                            nc.scalar.copy(
                                pt[:, b0:b0 + bw, :], ptp[:, :bw * P])
                        nb += 1
                    for j in range(nkb):
                        nc.tensor.matmul(
                            pav[:, h * D:h * D + D], pt[:, j, :],
                            v_sb[:, kj0 + j, :],
                            start=(j == 0), stop=(j == nkb - 1))
                    nc.vector.tensor_scalar_mul(
                        at_qi[:, h * D:h * D + D], pav[:, h * D:h * D + D],
                        recip[:])

                # attnT for this token block
                att = psA.tile([P, 512], f16, tag="bank", name=f"att{qi}")
                for m in range(4):
                    nc.tensor.transpose(
                        att[:, m * P:(m + 1) * P], at_qi[:, m * P:(m + 1) * P],
                        ident16[:])
                atT = att3.tile([P, 4, P], f16, tag="atT", name=f"atT{qi}")
                nc.scalar.copy(atT[:], att[:])
                return (atT, qi * P)

            # ---------------- merged pipeline ----------------
            prev = None
            for ts in range(NTC):
                if ts == 0:
                    ht = ht0
                else:
                    ht = hpool.tile([P, KC, TCW], f16, tag="ht", name=f"ht{ts}")
                    nc.sync.dma_start(ht[:], hTr[:, :, ts * TCW:(ts + 1) * TCW])
                proj_chunk(ts, ht)
                for qi in range(4 * ts, 4 * ts + 4):
                    prev = attn_block(qi, prev)
            emit_wo(prev)

    nc.compile()
    return nc


def _get_nc():
    if "nc" not in _CACHED:
        _CACHED["nc"] = _build_nc()
    return _CACHED["nc"]


def kernel(hidden_states, Wq, Wk, Wv, Wo, position_ids):
    from concourse.bass_utils import run_bass_kernel_spmd

    hidden_states = np.asarray(hidden_states)
    Wq, Wk, Wv, Wo = (np.asarray(a) for a in (Wq, Wk, Wv, Wo))
    position_ids = np.asarray(position_ids)

    inv_freq = 1.0 / (THETA ** (np.arange(0, D, 2, dtype=np.float64) / D))
    freqs = position_ids.astype(np.float64)[None, :] * inv_freq[:, None]
    cos_t = np.cos(freqs).astype(np.float16)
    sin_t = np.sin(freqs).astype(np.float16)

    in_maps = []
    for c in range(8):
        b, r = divmod(c, 4)
        in_maps.append({
            "hT": np.ascontiguousarray(hidden_states[b].T).astype(np.float16),
            "wqT": np.ascontiguousarray(Wq[512 * r:512 * (r + 1)].T).astype(np.float16),
            "wkT": np.ascontiguousarray(Wk[256 * r:256 * (r + 1)].T).astype(np.float16),
            "wvT": np.ascontiguousarray(Wv[256 * r:256 * (r + 1)].T).astype(np.float16),
            "woT": np.ascontiguousarray(Wo[:, 512 * r:512 * (r + 1)].T).astype(np.float16),
            "cosT": cos_t,
            "sinT": sin_t,
        })

    _CACHED["last_in_maps"] = in_maps
    globals()["_last_in_maps"] = in_maps
    res = run_bass_kernel_spmd(_get_nc(), in_maps, core_ids=list(range(8)))
    parts = [r["out"] for r in res.results]
    full = np.stack([
        parts[0] + parts[1] + parts[2] + parts[3],
        parts[4] + parts[5] + parts[6] + parts[7],
    ]).astype(np.float32)
    return full
